# revision 61
# baseline (speedup 1.0000x reference)
"""Additive noise channel kernel for 8 Trainium2 NeuronCores.

Computes out[b, s, 0:2] = complex_FIR(x, a)[b, s] + (L @ (scale * noise))[b, s]
with B=64, S=8192, T=129 taps, L lower-triangular [S, S].

Strategy ("fine8" mode)
-----------------------
The dominant cost is reading L (256 MB fp32, half zeros), so the kernel is
DMA-bound: every byte of L^T read is time on the (serialized, ~360 GB/s)
DMA device.  The output columns are sharded across cores in 16-column fine
strips (strip sigma covers columns [16*sigma, 16*sigma+16), core k owns
sigma == k mod 8).  With this interleave, core k's m-th strip always ends in
k-tile m, so the per-slot k-extents (m+1 tiles) are identical on every core:
the staircase of the triangular L packs with ZERO padding bytes -- 2080
128x16 tiles = 4.26 MB/core in fp8 (vs 4.72 MB for the 128-wide-strip
staircase, whose SPMD-uniform cover needs 28 padding k-tiles).

Noise k-tile t multiplies exactly the strips m >= t, which are contiguous
psum columns [16t, 1024) -- so each k-tile needs just one or two matmuls
(split at the 512-col psum bank boundary).  Both noise operands are fp8e4m3
and adjacent k-tiles are contracted pairwise in DoubleRow mode (0.5
cycles/row); the 16 leading columns of the even tile are covered by a tiny
single matmul, which also makes the pair's two tiles align exactly -- no
zero-padding blocks.  The runtime SNR scale folds into the host-packed L^T
(lt = 64*scale*L^T) and tap Toeplitz (a2 = 64*a), so one compile-time 1/64
rescale at PSUM evacuation restores units.

The FIR keeps the COARSE 128-column-strip sharding (its x-window locality
breaks under fine interleave: fine strips would need ~4x the x bytes), so
its column set differs from the noise shard's.  The two parts are therefore
stored separately -- FIR fp16, noise fp8e4m3 (noise is a small additive
component of the output, so fp8 storage costs ~2e-3 relative error) -- and
summed on the host during unsharding, which is free.

x is fp8e3m4 (4 mantissa bits; the FIR dominates output scale, so this is
the error-budget-limited choice: ~1.3e-2 relative, gate 2e-2), taps fp16.

Schedule: the lt stream is 10 large chunks -- every DMA instruction costs
~1.3us of serialized dispatch (SEQ config + shared HWDGE descriptor-gen),
so few large transfers keep dispatch off the critical path, and per-queue
in-order dispatch means tail stores must share the last chunk's queue to
not displace it on the device.  The noise psum lives in THREE tiles cut at
write-frontier boundaries (last writers: pair 15 / 26 / 31): dependency
tracking is tile-granular, so this is what lets each region evacuate right
after its last writer without blocking later matmuls (WAR) or parking a
PE-pipeline drain mid-ladder.  The scheduler coalesces all PE
engine-completion signals into one drain after the last matmul, so the two
tail evacuations run post-drain in parallel on Activation and DVE, split
at column 848 to balance their per-column rates.  The FIR store trails the
last chunk as a DMA-device filler while the final sem-prop -> matmul
ladder -> drain -> evacs -> dispatch chain (~3.0 us, the irreducible tail
of this structure) plays out.
"""

import os
import sys
import time

for _p in ("/opt/trn_rl_repo", "/root/.axon_site/_ro/trn_rl_repo"):
    if _p not in sys.path:
        sys.path.append(_p)

# the bass kernel executes through jax/PJRT on the axon-tunneled NeuronCores
os.environ.setdefault("JAX_PLATFORMS", "axon,cpu")

import numpy as np

import concourse.bass as bass
import concourse.mybir as mybir
import concourse.tile as tile
from concourse.tile import add_dep_helper
from concourse import bacc
from concourse.bass_utils import run_bass_kernel_spmd

B = 64          # batch
S = 8192        # block size
T = 129         # taps
H = (T - 1) // 2  # 64
P = 128         # partitions / k-tile
N_CORES = 8
N_SLOTS = 8     # coarse strips per core (FIR sharding)
W = 128         # coarse strip width
NKT = S // P    # 64 noise k-tiles
NPAIR = NKT // 2
FINE = 16       # fine strip width (noise sharding)
NFS = S // FINE // N_CORES   # 64 fine strips per core
COLS = NFS * FINE            # 1024 psum/output columns per core

C_LT = 64.0  # fp8 pre-scale; folded into lt (64*scale*L) and a2 (64*a)

# --- fine8 noise layout -----------------------------------------------------
# noise k-tile t feeds psum cols [16t, 1024): width w(t) = 1024 - 16t.
# DoubleRow pair p = tiles (2p, 2p+1): a 16-wide single matmul covers tile
# 2p's leading cols [32p, 32p+16); the DR matmul covers [32p+16, 1024) with
# i=0 -> tile 2p cols 16.., i=1 -> tile 2p+1 (exact alignment, no padding).
# Packed pair block per partition: [single16 | tile2p[16:] | tile2p+1] =
# 2*w(2p) - 16 bytes.


def _wp(p):
    return COLS - 32 * p


PAIR_BYTES = [2 * _wp(p) - 16 for p in range(NPAIR)]
LT_BYTES = sum(PAIR_BYTES)  # 33280 per partition

# lt DMA chunks: groups of pairs.  Large chunks keep the descriptor-gen
# device cold; the last chunk is a single tiny pair so the tail chain after
# the final bytes is minimal.
CHUNKS = [[0, 1], [2, 3], [4, 5], [6, 7], [8, 9, 10], [11, 12, 13],
          [14, 15, 16, 17], [18, 19, 20, 21, 22],
          [23, 24, 25, 26, 27], [28, 29, 30, 31]]
CHUNK_BYTES = [sum(PAIR_BYTES[p] for p in ch) for ch in CHUNKS]
CHUNK_OFF = np.cumsum([0] + CHUNK_BYTES).tolist()
MAX_CHUNK = max(CHUNK_BYTES)

NOISE_DT = "fine8"

LAST_RUN_SECONDS = None
_CACHE = {}


def _build_program(dt_name: str):
    assert dt_name == "fine8"
    f32 = mybir.dt.float32
    f16 = mybir.dt.float16
    f8 = mybir.dt.float8e4
    f8x = mybir.dt.float8e3
    DR = mybir.MatmulPerfMode.DoubleRow

    nc = bacc.Bacc("TRN2", target_bir_lowering=False, debug=False,
                   num_devices=N_CORES)

    lt = nc.dram_tensor("lt", [P * LT_BYTES], f8, kind="ExternalInput")
    npk = nc.dram_tensor("npk", [P, NKT, P], f8, kind="ExternalInput")
    fs = nc.dram_tensor("fs", [P, N_SLOTS * 2, P], f8x, kind="ExternalInput")
    a2 = nc.dram_tensor("a2", [P, 2, 2, P], f16, kind="ExternalInput")
    # separate outputs: FIR on the coarse shard (fp16), noise on the fine
    # shard (fp8); host sums them during unsharding.  row = plane*B + batch.
    fir_out = nc.dram_tensor("fir", [2 * B, N_SLOTS * W], f16,
                             kind="ExternalOutput")
    noise_out = nc.dram_tensor("noise", [2 * B, COLS], f8,
                               kind="ExternalOutput")

    with tile.TileContext(nc) as tc:
        with (
            tc.tile_pool(name="const", bufs=1) as const,
            tc.tile_pool(name="ltp", bufs=4) as ltp,
            tc.tile_pool(name="psum", bufs=1, space=bass.MemorySpace.PSUM) as psum,
            tc.tile_pool(name="stage", bufs=1) as stage,
        ):
            npk_sb = const.tile([P, NKT, P], f8)
            # first noise window leads the stream so pair-0 can start early
            nc.sync.dma_start(npk_sb[:, 0:16, :], npk.ap()[:, 0:16, :])
            fs_sb = const.tile([P, N_SLOTS * 2, P], f8x)
            a2_sb = const.tile([P, 2, 2, P], f16)
            fsi_sb = const.tile([P, N_SLOTS * 2, P], f16)

            # noise psum in THREE tiles cut at WAR boundaries: psA's last
            # writer is pair 15, psB1's pair 26, psB2's pair 31.  Evacuating
            # a tile after its last writer never blocks later matmuls (the
            # dependency tracker is tile-granular, so a read of a shared
            # tile would stall every later write to it).
            psA = psum.tile([P, 512], f32, name="npsA", tag="npsA")
            psB1 = psum.tile([P, 336], f32, name="npsB1", tag="npsB1")
            psB2 = psum.tile([P, 176], f32, name="npsB2", tag="npsB2")
            psF = [psum.tile([P, 512], f32, name=f"fps{i}", tag=f"fps{i}")
                   for i in range(2)]
            n_st = stage.tile([P, COLS], f8)
            f_st = stage.tile([P, N_SLOTS * W], f16)

            last_mm = [None]
            SEGS = ((0, 512, psA), (512, 848, psB1), (848, COLS, psB2))

            def seg_mm(lo, hi, lhsT, rhs3, perf, start, stops):
                """matmul into noise psum cols [lo, hi), split at the psum
                tile boundaries.  rhs3 is indexed in region-relative cols;
                stops = per-segment stop flags."""
                for (b0, b1, ps), stop in zip(SEGS, stops):
                    s0, s1 = max(lo, b0), min(hi, b1)
                    if s0 >= s1:
                        continue
                    r0, r1 = s0 - lo, s1 - lo
                    rhs = rhs3[:, :, r0:r1] if perf else rhs3[:, r0:r1]
                    last_mm[0] = nc.tensor.matmul(
                        ps[:, s0 - b0:s1 - b0],
                        lhsT, rhs, start=start, stop=stop,
                        perf_mode=perf, skip_group_check=True)

            def fir_mms(j):
                for sdx in (0, 1):
                    for c in (0, 1):
                        g = j * 2 + c
                        src = fs_sb if sdx == 0 else fsi_sb
                        nc.tensor.matmul(
                            psF[j // 4][:, 128 * (j % 4):128 * (j % 4) + 128],
                            src[:, g, :], a2_sb[:, sdx, c, :],
                            start=(sdx == 0 and c == 0),
                            stop=(sdx == 1 and c == 1),
                            skip_group_check=True)

            n_dma = 0
            done = 0
            for ci, pairs in enumerate(CHUNKS):
                cb = CHUNK_BYTES[ci]
                ltc = ltp.tile([P, MAX_CHUNK], f8, tag="lt", name=f"lt{ci}")
                dma_eng = nc.sync if n_dma % 2 == 0 else nc.scalar
                n_dma += 1
                chunk_inst = dma_eng.dma_start(
                    ltc[:, :cb],
                    lt.ap()[CHUNK_OFF[ci] * P:CHUNK_OFF[ci + 1] * P].rearrange(
                        "(p w) -> p w", p=P))
                # anchor dep-free const loads behind the chunk stream so the
                # scheduler can't hoist them ahead of the lt bytes
                if ci == 1:
                    fs_inst = nc.sync.dma_start(fs_sb[:], fs.ap())
                    add_dep_helper(fs_inst.ins, chunk_inst.ins, sync=False,
                                   reason="defer fs")
                if ci == 2:
                    a2_inst = dma_eng.dma_start(a2_sb[:], a2.ap())
                    add_dep_helper(a2_inst.ins, chunk_inst.ins, sync=False,
                                   reason="defer a2")
                    np2 = nc.sync.dma_start(npk_sb[:, 16:40, :],
                                            npk.ap()[:, 16:40, :])
                    add_dep_helper(np2.ins, chunk_inst.ins, sync=False,
                                   reason="defer npk2")
                    for g in range(N_SLOTS * 2):
                        nc.vector.tensor_scalar_mul(fsi_sb[:, g, 0:B],
                                                    fs_sb[:, g, B:2 * B], -1.0)
                        nc.vector.tensor_copy(fsi_sb[:, g, B:2 * B],
                                              fs_sb[:, g, 0:B])
                if ci == 5:
                    np3 = nc.scalar.dma_start(npk_sb[:, 40:64, :],
                                              npk.ap()[:, 40:64, :])
                    add_dep_helper(np3.ins, chunk_inst.ins, sync=False,
                                   reason="defer npk3")

                off = 0
                for p in pairs:
                    w = _wp(p)
                    # single: tile 2p leading 16 cols -> [32p, 32p+16)
                    seg_mm(32 * p, 32 * p + 16, npk_sb[:, 2 * p, :],
                           ltc[:, off:off + 16], None, start=(p == 0),
                           stops=(False, p == 26, False))
                    # DoubleRow: tiles (2p, 2p+1) -> [32p+16, 1024)
                    dr = ltc[:, off + 16:off + 16 + 2 * (w - 16)].rearrange(
                        "q (two w) -> q two w", two=2)
                    seg_mm(32 * p + 16, COLS, npk_sb[:, 2 * p:2 * p + 2, :],
                           dr, DR, start=(p == 0),
                           stops=(p == 15, False, p == NPAIR - 1))
                    if p == 26:
                        # emit psB1's evac right here so its PE-drain lands
                        # mid-chunk (hidden by the next pairs' DMA-sem
                        # window) instead of being coalesced into the final
                        # drain and dragged onto the tail chain
                        nc.scalar.activation(
                            n_st[:, 512:848], psB1[:],
                            mybir.ActivationFunctionType.Copy,
                            scale=1.0 / C_LT)
                    off += PAIR_BYTES[p]

                if ci == 3:
                    for j in range(N_SLOTS):
                        fir_mms(j)
                    for i in range(2):
                        nc.vector.tensor_scalar_mul(
                            f_st[:, 512 * i:512 * (i + 1)], psF[i][:],
                            1.0 / C_LT)

                # evacuate each noise psum tile once, right after its LAST
                # writer's chunk: psA after pair 15 (chunk 6), psB1 after
                # pair 27 (chunk 8), psB2 after pair 31 (last chunk).  This
                # gives three PE-drain points that land between chunk
                # ladders, and no evac ever blocks a later matmul.
                # mid-stream evacs ride the Activation engine so the DVE is
                # free the moment the final evac's gate opens
                if ci == 6:
                    nc.scalar.activation(n_st[:, 0:512], psA[:],
                                         mybir.ActivationFunctionType.Copy,
                                         scale=1.0 / C_LT)

                if ci == len(CHUNKS) - 1:
                    # tail fillers on the last chunk's queue (cannot overtake
                    # it on the DMA device), keeping the device busy during
                    # the final sem -> matmul -> evac -> store chain
                    st1 = dma_eng.dma_start(fir_out.ap(), f_st[:])
                    add_dep_helper(st1.ins, chunk_inst.ins, sync=False,
                                   reason="tail filler fir")
                    st2 = dma_eng.dma_start(noise_out.ap()[:, 0:512],
                                            n_st[:, 0:512])
                    add_dep_helper(st2.ins, chunk_inst.ins, sync=False,
                                   reason="tail filler noiseA")
                    ev1 = nc.vector.tensor_scalar_mul(
                        n_st[:, 848:COLS], psB2[:], 1.0 / C_LT)
                    add_dep_helper(ev1.ins, last_mm[0].ins, sync=True,
                                   reason="final evac after all matmuls")


            nc.sync.dma_start(noise_out.ap()[:, 512:COLS],
                              n_st[:, 512:COLS])

    nc.compile()
    return nc


def _sbuf_image(arr_ktpm):
    """[nkt*128, m] k-tile-major -> SBUF image [128, nkt*m]."""
    nktp, m = arr_ktpm.shape
    nkt = nktp // P
    return np.ascontiguousarray(
        arr_ktpm.reshape(nkt, P, m).transpose(1, 0, 2).reshape(P, nkt * m))


def _prep_inputs(x_real, x_imag, a_real, a_imag, L, noise_r, noise_i, N0,
                 dt_name: str):
    import ml_dtypes
    f8 = ml_dtypes.float8_e4m3

    scale = np.float32(np.sqrt(0.5 * np.power(10.0, np.float64(N0[0]) / 10.0)))
    lt_scale = np.float32(C_LT) * scale

    # packed raw noise [S, 128]: cols 0:64 real, 64:128 imag (e4m3)
    npkf = np.empty((S, 2 * B), np.float32)
    npkf[:, :B] = noise_r.T
    npkf[:, B:] = noise_i.T
    npk = _sbuf_image(npkf.astype(f8)).reshape(P, NKT, P)

    # x transposed, zero-padded by H: row r <-> x col r - H
    xpad = np.zeros((S + 2 * H, 2 * B), np.float32)
    xpad[H:H + S, :B] = x_real.T
    xpad[H:H + S, B:] = x_imag.T
    xpad = xpad.astype(ml_dtypes.float8_e3m4)

    # banded Toeplitz of the taps, pre-scaled by C_LT
    a2 = np.zeros((2, 2 * P, P), np.float32)
    rr = np.arange(2 * P)[:, None]
    jj = np.arange(W)[None, :]
    tap_idx = jj + 2 * H - rr
    valid = (tap_idx >= 0) & (tap_idx < T)
    a2[0][valid] = C_LT * np.asarray(a_real, np.float32)[tap_idx[valid]]
    a2[1][valid] = C_LT * np.asarray(a_imag, np.float32)[tap_idx[valid]]
    a2 = _sbuf_image(a2.reshape(2 * 2 * P, P).astype(np.float16)).reshape(
        P, 2, 2, P)

    L = np.asarray(L, np.float32)

    in_maps = []
    for k in range(N_CORES):
        # fine-strip L^T stream: tile t = L^T[128t:128t+128, cols of strips
        # m >= t], strips m -> global cols 128m+16k+[0,16)
        tiles = []
        for t in range(NKT):
            cols = (128 * np.arange(t, NFS)[:, None] + 16 * k
                    + np.arange(FINE)[None, :]).ravel()
            blk = (lt_scale * L[cols, 128 * t:128 * (t + 1)].T).astype(f8)
            tiles.append(np.ascontiguousarray(blk))   # [128, 16*(64-t)]
        stream = np.empty((P, LT_BYTES), f8)
        off = 0
        for p in range(NPAIR):
            w = _wp(p)
            stream[:, off:off + 16] = tiles[2 * p][:, :16]
            stream[:, off + 16:off + w] = tiles[2 * p][:, 16:]
            stream[:, off + w:off + 2 * w - 16] = tiles[2 * p + 1]
            off += PAIR_BYTES[p]
        assert off == LT_BYTES
        # flatten chunk-by-chunk so each chunk is contiguous in DRAM
        ltpack = np.concatenate(
            [stream[:, CHUNK_OFF[ci]:CHUNK_OFF[ci + 1]].ravel()
             for ci in range(len(CHUNKS))])

        # coarse-strip x windows for the FIR (identical to the 128-col shard)
        fsk = np.empty((N_SLOTS * 2, P, 2 * B), ml_dtypes.float8_e3m4)
        for j in range(N_SLOTS):
            s0 = P * (8 * j + k)
            fsk[j * 2] = xpad[s0:s0 + P]
            fsk[j * 2 + 1] = xpad[s0 + P:s0 + 2 * P]
        fsk = _sbuf_image(fsk.reshape(N_SLOTS * 2 * P, 2 * B)).reshape(
            P, N_SLOTS * 2, P)
        in_maps.append({"lt": ltpack, "npk": npk, "fs": fsk, "a2": a2})
    return in_maps


def kernel(x_real, x_imag, a_real, a_imag, L, noise_r, noise_i, N0):
    global LAST_RUN_SECONDS
    inputs = dict(x_real=np.asarray(x_real, np.float32),
                  x_imag=np.asarray(x_imag, np.float32),
                  a_real=np.asarray(a_real, np.float32),
                  a_imag=np.asarray(a_imag, np.float32),
                  L=np.asarray(L, np.float32),
                  noise_r=np.asarray(noise_r, np.float32),
                  noise_i=np.asarray(noise_i, np.float32),
                  N0=np.asarray(N0, np.float32))

    if NOISE_DT not in _CACHE:
        _CACHE[NOISE_DT] = _build_program(NOISE_DT)
    nc = _CACHE[NOISE_DT]

    in_maps = _prep_inputs(**inputs, dt_name=NOISE_DT)

    t0 = time.time()
    res = run_bass_kernel_spmd(nc, in_maps, core_ids=list(range(N_CORES)))
    LAST_RUN_SECONDS = time.time() - t0

    full = np.zeros((2, B, S), np.float32)
    for k in range(N_CORES):
        fir = np.asarray(res.results[k]["fir"],
                         np.float32).reshape(2, B, N_SLOTS, W)
        # coarse: slot j -> cols [128*(8j+k), +128)
        fir_view = full.reshape(2, B, N_SLOTS, N_CORES, W)
        fir_view[:, :, :, k, :] += fir
        noi = np.asarray(res.results[k]["noise"],
                         np.float32).reshape(2, B, NFS, FINE)
        # fine: strip m -> cols 128m + 16k + [0,16)
        noi_view = full.reshape(2, B, NFS, N_CORES, FINE)
        noi_view[:, :, :, k, :] += noi
    out = np.empty((B, S, 2), np.float32)
    out[:, :, 0] = full[0]
    out[:, :, 1] = full[1]
    return out


# revision 66
# speedup vs baseline: 1.0024x; 1.0024x over previous
"""Additive noise channel kernel for 8 Trainium2 NeuronCores.

Computes out[b, s, 0:2] = complex_FIR(x, a)[b, s] + (L @ (scale * noise))[b, s]
with B=64, S=8192, T=129 taps, L lower-triangular [S, S].

Strategy ("fine8" mode)
-----------------------
The dominant cost is reading L (256 MB fp32, half zeros), so the kernel is
DMA-bound: every byte of L^T read is time on the (serialized, ~360 GB/s)
DMA device.  The output columns are sharded across cores in 16-column fine
strips (strip sigma covers columns [16*sigma, 16*sigma+16), core k owns
sigma == k mod 8).  With this interleave, core k's m-th strip always ends in
k-tile m, so the per-slot k-extents (m+1 tiles) are identical on every core:
the staircase of the triangular L packs with ZERO padding bytes -- 2080
128x16 tiles = 4.26 MB/core in fp8 (vs 4.72 MB for the 128-wide-strip
staircase, whose SPMD-uniform cover needs 28 padding k-tiles).

Noise k-tile t multiplies exactly the strips m >= t, which are contiguous
psum columns [16t, 1024) -- so each k-tile needs just one or two matmuls
(split at the 512-col psum bank boundary).  Both noise operands are fp8e4m3
and adjacent k-tiles are contracted pairwise in DoubleRow mode (0.5
cycles/row); the 16 leading columns of the even tile are covered by a tiny
single matmul, which also makes the pair's two tiles align exactly -- no
zero-padding blocks.  The runtime SNR scale folds into the host-packed L^T
(lt = 64*scale*L^T) and tap Toeplitz (a2 = 64*a), so one compile-time 1/64
rescale at PSUM evacuation restores units.

The FIR keeps the COARSE 128-column-strip sharding (its x-window locality
breaks under fine interleave: fine strips would need ~4x the x bytes), so
its column set differs from the noise shard's.  The two parts are therefore
stored separately -- FIR fp16, noise fp8e4m3 (noise is a small additive
component of the output, so fp8 storage costs ~2e-3 relative error) -- and
summed on the host during unsharding, which is free.

x is fp8e3m4 (4 mantissa bits; the FIR dominates output scale, so this is
the error-budget-limited choice: ~1.3e-2 relative, gate 2e-2), taps fp16.

Schedule: the lt stream is 10 large chunks -- every DMA instruction costs
~1.3us of serialized dispatch (SEQ config + shared HWDGE descriptor-gen),
so few large transfers keep dispatch off the critical path, and per-queue
in-order dispatch means tail stores must share the last chunk's queue to
not displace it on the device.  The noise psum lives in THREE tiles cut at
write-frontier boundaries (last writers: pair 15 / 26 / 31): dependency
tracking is tile-granular, so this is what lets each region evacuate right
after its last writer without blocking later matmuls (WAR) or parking a
PE-pipeline drain mid-ladder.  The scheduler coalesces all PE
engine-completion signals into one drain after the last matmul, so the two
tail evacuations run post-drain in parallel on Activation and DVE, split
at column 848 to balance their per-column rates.  The FIR store trails the
last chunk as a DMA-device filler while the final sem-prop -> matmul
ladder -> drain -> evacs -> dispatch chain (~3.0 us, the irreducible tail
of this structure) plays out.
"""

import os
import sys
import time

for _p in ("/opt/trn_rl_repo", "/root/.axon_site/_ro/trn_rl_repo"):
    if _p not in sys.path:
        sys.path.append(_p)

# the bass kernel executes through jax/PJRT on the axon-tunneled NeuronCores
os.environ.setdefault("JAX_PLATFORMS", "axon,cpu")

import numpy as np

import concourse.bass as bass
import concourse.mybir as mybir
import concourse.tile as tile
from concourse.tile import add_dep_helper
from concourse import bacc
from concourse.bass_utils import run_bass_kernel_spmd

B = 64          # batch
S = 8192        # block size
T = 129         # taps
H = (T - 1) // 2  # 64
P = 128         # partitions / k-tile
N_CORES = 8
N_SLOTS = 8     # coarse strips per core (FIR sharding)
W = 128         # coarse strip width
NKT = S // P    # 64 noise k-tiles
NPAIR = NKT // 2
FINE = 16       # fine strip width (noise sharding)
NFS = S // FINE // N_CORES   # 64 fine strips per core
COLS = NFS * FINE            # 1024 psum/output columns per core

C_LT = 64.0  # fp8 pre-scale; folded into lt (64*scale*L) and a2 (64*a)

# --- fine8 noise layout -----------------------------------------------------
# noise k-tile t feeds psum cols [16t, 1024): width w(t) = 1024 - 16t.
# DoubleRow pair p = tiles (2p, 2p+1): a 16-wide single matmul covers tile
# 2p's leading cols [32p, 32p+16); the DR matmul covers [32p+16, 1024) with
# i=0 -> tile 2p cols 16.., i=1 -> tile 2p+1 (exact alignment, no padding).
# Packed pair block per partition: [single16 | tile2p[16:] | tile2p+1] =
# 2*w(2p) - 16 bytes.


def _wp(p):
    return COLS - 32 * p


PAIR_BYTES = [2 * _wp(p) - 16 for p in range(NPAIR)]
LT_BYTES = sum(PAIR_BYTES)  # 33280 per partition

# lt DMA chunks: groups of pairs.  Large chunks keep the descriptor-gen
# device cold; the last chunk is a single tiny pair so the tail chain after
# the final bytes is minimal.
CHUNKS = [[0, 1], [2, 3], [4, 5], [6, 7], [8, 9, 10], [11, 12, 13, 14],
          [15, 16, 17, 18], [19, 20, 21, 22, 23],
          [24, 25, 26, 27], [28, 29, 30, 31]]
CHUNK_BYTES = [sum(PAIR_BYTES[p] for p in ch) for ch in CHUNKS]
CHUNK_OFF = np.cumsum([0] + CHUNK_BYTES).tolist()
MAX_CHUNK = max(CHUNK_BYTES)

NOISE_DT = "fine8"

LAST_RUN_SECONDS = None
_CACHE = {}


def _build_program(dt_name: str):
    assert dt_name == "fine8"
    f32 = mybir.dt.float32
    f16 = mybir.dt.float16
    f8 = mybir.dt.float8e4
    f8x = mybir.dt.float8e3
    DR = mybir.MatmulPerfMode.DoubleRow

    nc = bacc.Bacc("TRN2", target_bir_lowering=False, debug=False,
                   num_devices=N_CORES)

    lt = nc.dram_tensor("lt", [P * LT_BYTES], f8, kind="ExternalInput")
    npk = nc.dram_tensor("npk", [P, NKT, P], f8, kind="ExternalInput")
    fs = nc.dram_tensor("fs", [P, N_SLOTS * 2, P], f8x, kind="ExternalInput")
    a2 = nc.dram_tensor("a2", [P, 2, 2, P], f16, kind="ExternalInput")
    # separate outputs: FIR on the coarse shard (fp16), noise on the fine
    # shard (fp8); host sums them during unsharding.  row = plane*B + batch.
    fir_out = nc.dram_tensor("fir", [2 * B, N_SLOTS * W], f16,
                             kind="ExternalOutput")
    noise_out = nc.dram_tensor("noise", [2 * B, COLS], f8,
                               kind="ExternalOutput")

    with tile.TileContext(nc) as tc:
        with (
            tc.tile_pool(name="const", bufs=1) as const,
            tc.tile_pool(name="ltp", bufs=4) as ltp,
            tc.tile_pool(name="psum", bufs=1, space=bass.MemorySpace.PSUM) as psum,
            tc.tile_pool(name="stage", bufs=1) as stage,
        ):
            npk_sb = const.tile([P, NKT, P], f8)
            # first noise window leads the stream so pair-0 can start early
            nc.sync.dma_start(npk_sb[:, 0:16, :], npk.ap()[:, 0:16, :])
            fs_sb = const.tile([P, N_SLOTS * 2, P], f8x)
            a2_sb = const.tile([P, 2, 2, P], f16)
            fsi_sb = const.tile([P, N_SLOTS * 2, P], f16)

            # noise psum in THREE tiles cut at WAR boundaries: psA's last
            # writer is pair 15, psB1's pair 26, psB2's pair 31.  Evacuating
            # a tile after its last writer never blocks later matmuls (the
            # dependency tracker is tile-granular, so a read of a shared
            # tile would stall every later write to it).
            psA = psum.tile([P, 512], f32, name="npsA", tag="npsA")
            psB1 = psum.tile([P, 336], f32, name="npsB1", tag="npsB1")
            psB2 = psum.tile([P, 176], f32, name="npsB2", tag="npsB2")
            psF = [psum.tile([P, 512], f32, name=f"fps{i}", tag=f"fps{i}")
                   for i in range(2)]
            n_st = stage.tile([P, COLS], f8)
            f_st = stage.tile([P, N_SLOTS * W], f16)

            last_mm = [None]
            SEGS = ((0, 512, psA), (512, 848, psB1), (848, COLS, psB2))

            def seg_mm(lo, hi, lhsT, rhs3, perf, start, stops):
                """matmul into noise psum cols [lo, hi), split at the psum
                tile boundaries.  rhs3 is indexed in region-relative cols;
                stops = per-segment stop flags."""
                for (b0, b1, ps), stop in zip(SEGS, stops):
                    s0, s1 = max(lo, b0), min(hi, b1)
                    if s0 >= s1:
                        continue
                    r0, r1 = s0 - lo, s1 - lo
                    rhs = rhs3[:, :, r0:r1] if perf else rhs3[:, r0:r1]
                    last_mm[0] = nc.tensor.matmul(
                        ps[:, s0 - b0:s1 - b0],
                        lhsT, rhs, start=start, stop=stop,
                        perf_mode=perf, skip_group_check=True)

            def fir_mms(j):
                for sdx in (0, 1):
                    for c in (0, 1):
                        g = j * 2 + c
                        src = fs_sb if sdx == 0 else fsi_sb
                        nc.tensor.matmul(
                            psF[j // 4][:, 128 * (j % 4):128 * (j % 4) + 128],
                            src[:, g, :], a2_sb[:, sdx, c, :],
                            start=(sdx == 0 and c == 0),
                            stop=(sdx == 1 and c == 1),
                            skip_group_check=True)

            n_dma = 0
            done = 0
            for ci, pairs in enumerate(CHUNKS):
                cb = CHUNK_BYTES[ci]
                ltc = ltp.tile([P, MAX_CHUNK], f8, tag="lt", name=f"lt{ci}")
                dma_eng = nc.sync if n_dma % 2 == 0 else nc.scalar
                n_dma += 1
                chunk_inst = dma_eng.dma_start(
                    ltc[:, :cb],
                    lt.ap()[CHUNK_OFF[ci] * P:CHUNK_OFF[ci + 1] * P].rearrange(
                        "(p w) -> p w", p=P))
                # anchor dep-free const loads behind the chunk stream so the
                # scheduler can't hoist them ahead of the lt bytes
                if ci == 1:
                    fs_inst = nc.sync.dma_start(fs_sb[:], fs.ap())
                    add_dep_helper(fs_inst.ins, chunk_inst.ins, sync=False,
                                   reason="defer fs")
                if ci == 2:
                    a2_inst = dma_eng.dma_start(a2_sb[:], a2.ap())
                    add_dep_helper(a2_inst.ins, chunk_inst.ins, sync=False,
                                   reason="defer a2")
                    np2 = nc.sync.dma_start(npk_sb[:, 16:40, :],
                                            npk.ap()[:, 16:40, :])
                    add_dep_helper(np2.ins, chunk_inst.ins, sync=False,
                                   reason="defer npk2")
                    for g in range(N_SLOTS * 2):
                        nc.vector.tensor_scalar_mul(fsi_sb[:, g, 0:B],
                                                    fs_sb[:, g, B:2 * B], -1.0)
                        nc.vector.tensor_copy(fsi_sb[:, g, B:2 * B],
                                              fs_sb[:, g, 0:B])
                if ci == 5:
                    np3 = nc.scalar.dma_start(npk_sb[:, 40:64, :],
                                              npk.ap()[:, 40:64, :])
                    add_dep_helper(np3.ins, chunk_inst.ins, sync=False,
                                   reason="defer npk3")

                off = 0
                for p in pairs:
                    w = _wp(p)
                    # single: tile 2p leading 16 cols -> [32p, 32p+16)
                    seg_mm(32 * p, 32 * p + 16, npk_sb[:, 2 * p, :],
                           ltc[:, off:off + 16], None, start=(p == 0),
                           stops=(False, p == 26, False))
                    # DoubleRow: tiles (2p, 2p+1) -> [32p+16, 1024)
                    dr = ltc[:, off + 16:off + 16 + 2 * (w - 16)].rearrange(
                        "q (two w) -> q two w", two=2)
                    seg_mm(32 * p + 16, COLS, npk_sb[:, 2 * p:2 * p + 2, :],
                           dr, DR, start=(p == 0),
                           stops=(p == 15, False, p == NPAIR - 1))
                    if p == 26:
                        # emit psB1's evac right here so its PE-drain lands
                        # mid-chunk (hidden by the next pairs' DMA-sem
                        # window) instead of being coalesced into the final
                        # drain and dragged onto the tail chain
                        nc.scalar.activation(
                            n_st[:, 512:848], psB1[:],
                            mybir.ActivationFunctionType.Copy,
                            scale=1.0 / C_LT)
                    off += PAIR_BYTES[p]

                if ci == 3:
                    for j in range(N_SLOTS):
                        fir_mms(j)
                    for i in range(2):
                        nc.vector.tensor_scalar_mul(
                            f_st[:, 512 * i:512 * (i + 1)], psF[i][:],
                            1.0 / C_LT)

                # evacuate each noise psum tile once, right after its LAST
                # writer's chunk: psA after pair 15 (chunk 6), psB1 after
                # pair 27 (chunk 8), psB2 after pair 31 (last chunk).  This
                # gives three PE-drain points that land between chunk
                # ladders, and no evac ever blocks a later matmul.
                # mid-stream evacs ride the Activation engine so the DVE is
                # free the moment the final evac's gate opens
                if ci == 6:
                    nc.scalar.activation(n_st[:, 0:512], psA[:],
                                         mybir.ActivationFunctionType.Copy,
                                         scale=1.0 / C_LT)

                if ci == len(CHUNKS) - 1:
                    # tail fillers on the last chunk's queue (cannot overtake
                    # it on the DMA device), keeping the device busy during
                    # the final sem -> matmul -> evac -> store chain
                    st1 = dma_eng.dma_start(fir_out.ap(), f_st[:])
                    add_dep_helper(st1.ins, chunk_inst.ins, sync=False,
                                   reason="tail filler fir")
                    st2 = dma_eng.dma_start(noise_out.ap()[:, 0:512],
                                            n_st[:, 0:512])
                    add_dep_helper(st2.ins, chunk_inst.ins, sync=False,
                                   reason="tail filler noiseA")
                    ev1 = nc.vector.tensor_scalar_mul(
                        n_st[:, 848:COLS], psB2[:], 1.0 / C_LT)
                    add_dep_helper(ev1.ins, last_mm[0].ins, sync=True,
                                   reason="final evac after all matmuls")


            nc.sync.dma_start(noise_out.ap()[:, 512:COLS],
                              n_st[:, 512:COLS])

    nc.compile()
    return nc


def _sbuf_image(arr_ktpm):
    """[nkt*128, m] k-tile-major -> SBUF image [128, nkt*m]."""
    nktp, m = arr_ktpm.shape
    nkt = nktp // P
    return np.ascontiguousarray(
        arr_ktpm.reshape(nkt, P, m).transpose(1, 0, 2).reshape(P, nkt * m))


def _prep_inputs(x_real, x_imag, a_real, a_imag, L, noise_r, noise_i, N0,
                 dt_name: str):
    import ml_dtypes
    f8 = ml_dtypes.float8_e4m3

    scale = np.float32(np.sqrt(0.5 * np.power(10.0, np.float64(N0[0]) / 10.0)))
    lt_scale = np.float32(C_LT) * scale

    # packed raw noise [S, 128]: cols 0:64 real, 64:128 imag (e4m3)
    npkf = np.empty((S, 2 * B), np.float32)
    npkf[:, :B] = noise_r.T
    npkf[:, B:] = noise_i.T
    npk = _sbuf_image(npkf.astype(f8)).reshape(P, NKT, P)

    # x transposed, zero-padded by H: row r <-> x col r - H
    xpad = np.zeros((S + 2 * H, 2 * B), np.float32)
    xpad[H:H + S, :B] = x_real.T
    xpad[H:H + S, B:] = x_imag.T
    xpad = xpad.astype(ml_dtypes.float8_e3m4)

    # banded Toeplitz of the taps, pre-scaled by C_LT
    a2 = np.zeros((2, 2 * P, P), np.float32)
    rr = np.arange(2 * P)[:, None]
    jj = np.arange(W)[None, :]
    tap_idx = jj + 2 * H - rr
    valid = (tap_idx >= 0) & (tap_idx < T)
    a2[0][valid] = C_LT * np.asarray(a_real, np.float32)[tap_idx[valid]]
    a2[1][valid] = C_LT * np.asarray(a_imag, np.float32)[tap_idx[valid]]
    a2 = _sbuf_image(a2.reshape(2 * 2 * P, P).astype(np.float16)).reshape(
        P, 2, 2, P)

    L = np.asarray(L, np.float32)

    in_maps = []
    for k in range(N_CORES):
        # fine-strip L^T stream: tile t = L^T[128t:128t+128, cols of strips
        # m >= t], strips m -> global cols 128m+16k+[0,16)
        tiles = []
        for t in range(NKT):
            cols = (128 * np.arange(t, NFS)[:, None] + 16 * k
                    + np.arange(FINE)[None, :]).ravel()
            blk = (lt_scale * L[cols, 128 * t:128 * (t + 1)].T).astype(f8)
            tiles.append(np.ascontiguousarray(blk))   # [128, 16*(64-t)]
        stream = np.empty((P, LT_BYTES), f8)
        off = 0
        for p in range(NPAIR):
            w = _wp(p)
            stream[:, off:off + 16] = tiles[2 * p][:, :16]
            stream[:, off + 16:off + w] = tiles[2 * p][:, 16:]
            stream[:, off + w:off + 2 * w - 16] = tiles[2 * p + 1]
            off += PAIR_BYTES[p]
        assert off == LT_BYTES
        # flatten chunk-by-chunk so each chunk is contiguous in DRAM
        ltpack = np.concatenate(
            [stream[:, CHUNK_OFF[ci]:CHUNK_OFF[ci + 1]].ravel()
             for ci in range(len(CHUNKS))])

        # coarse-strip x windows for the FIR (identical to the 128-col shard)
        fsk = np.empty((N_SLOTS * 2, P, 2 * B), ml_dtypes.float8_e3m4)
        for j in range(N_SLOTS):
            s0 = P * (8 * j + k)
            fsk[j * 2] = xpad[s0:s0 + P]
            fsk[j * 2 + 1] = xpad[s0 + P:s0 + 2 * P]
        fsk = _sbuf_image(fsk.reshape(N_SLOTS * 2 * P, 2 * B)).reshape(
            P, N_SLOTS * 2, P)
        in_maps.append({"lt": ltpack, "npk": npk, "fs": fsk, "a2": a2})
    return in_maps


def kernel(x_real, x_imag, a_real, a_imag, L, noise_r, noise_i, N0):
    global LAST_RUN_SECONDS
    inputs = dict(x_real=np.asarray(x_real, np.float32),
                  x_imag=np.asarray(x_imag, np.float32),
                  a_real=np.asarray(a_real, np.float32),
                  a_imag=np.asarray(a_imag, np.float32),
                  L=np.asarray(L, np.float32),
                  noise_r=np.asarray(noise_r, np.float32),
                  noise_i=np.asarray(noise_i, np.float32),
                  N0=np.asarray(N0, np.float32))

    if NOISE_DT not in _CACHE:
        _CACHE[NOISE_DT] = _build_program(NOISE_DT)
    nc = _CACHE[NOISE_DT]

    in_maps = _prep_inputs(**inputs, dt_name=NOISE_DT)

    t0 = time.time()
    res = run_bass_kernel_spmd(nc, in_maps, core_ids=list(range(N_CORES)))
    LAST_RUN_SECONDS = time.time() - t0

    full = np.zeros((2, B, S), np.float32)
    for k in range(N_CORES):
        fir = np.asarray(res.results[k]["fir"],
                         np.float32).reshape(2, B, N_SLOTS, W)
        # coarse: slot j -> cols [128*(8j+k), +128)
        fir_view = full.reshape(2, B, N_SLOTS, N_CORES, W)
        fir_view[:, :, :, k, :] += fir
        noi = np.asarray(res.results[k]["noise"],
                         np.float32).reshape(2, B, NFS, FINE)
        # fine: strip m -> cols 128m + 16k + [0,16)
        noi_view = full.reshape(2, B, NFS, N_CORES, FINE)
        noi_view[:, :, :, k, :] += noi
    out = np.empty((B, S, 2), np.float32)
    out[:, :, 0] = full[0]
    out[:, :, 1] = full[1]
    return out
